# revision 41
# baseline (speedup 1.0000x reference)
"""Distributed GQA attention kernel for 8 TRN2 NeuronCores (Bass/Tile).

Problem (hardcoded): B=2, S=2048, DM=1024, H=16 q-heads, KH=4 kv-heads, HD=64.
reference: out = softmax_causal((RoPE(x@wq) @ RoPE(x@wk)^T)/sqrt(HD)) @ (x@wv) @ wo

Sharding: core c in 0..7 -> batch b = c//4, kv-group g = c%4.
Each core computes q-heads [4g..4g+4) and kv head g for its batch, keeps the
normalized attention probabilities in SBUF, row-shards the o-projection
(wo rows [256g:256g+256)), and the 4 cores of one batch ReduceScatter(add)
their full-width o-proj partials.  Core g ends up with token rows
[256g:256g+256) of each 1024-token half; the host reassembles the full
[2,2048,1024] output from the 8 per-core [512,1024] slices.

The ReduceScatter replaces the baseline AllGather + column-sharded o-proj:
the collective cost scales with its *output* size, so RS (0.5-2 MiB out)
beats AG (4 MiB out), and splitting it per 1024-token half lets the first
RS run concurrently with the second half's attention compute.

All matmuls run in bf16 with f32 PSUM accumulation.  Scores are computed
transposed ([k,q]) so the softmax denominator falls out of a ones-column in
the PV matmul; softmax skips max-subtraction (scores are O(3) for this
problem scale, well within fp32 exp range).  RoPE's rotate_half is a
permutation matmul; causality is handled by issuing score matmuls only for
q >= k plus one triangular mask multiply on diagonal 128x128 blocks.

_build(nrep=N) repeats the whole per-iteration body N times inside one NEFF
(used by the benchmark to amortize the ~80 ms axon dispatch overhead and
measure per-iteration HW time from the slope).
"""

import numpy as np
import ml_dtypes

import concourse.bass as bass
import concourse.bacc as bacc
import concourse.mybir as mybir
import concourse.tile as tile
from concourse import bass_utils

B, S, DM = 2, 2048, 1024
H, KH, HD = 16, 4, 64
NCORES = 8
TPG = 4            # tensor-parallel group size (cores per batch)
QH_PER_CORE = 4    # q heads per core
QR = QH_PER_CORE * HD   # 256 q rows per core
ORS = S // TPG     # 512 out token rows per core (256 per half)

F32 = mybir.dt.float32
BF16 = mybir.dt.bfloat16
FP = mybir.ActivationFunctionType

_CACHE = {}
PROFILE = False
LAST_RESULTS = None

RS_BF16 = False   # ReduceScatter payload dtype (False -> f32)
XB2 = False       # double-buffer x/q/k/v tiles for cross-iteration overlap
SCB_CFG = 3       # "sc" psum slots
ATB_CFG = 1       # "attn" psum slots
PEND = 4          # PV lookahead depth
PTB = 6           # pt buffer count
TRI_POOL = False  # tri-mask multiplies on gpsimd instead of DVE
PBCAST = False    # zr broadcast via gpsimd.partition_broadcast (no DMA)
ZBC_PE = False    # 1/Z broadcast via PE ones-column matmul (no DMA roundtrip)


def _build(nrep=1):
    nc = bacc.Bacc("TRN2", debug=False, enable_asserts=False,
                   num_devices=NCORES)

    xT = nc.dram_tensor("xT", [DM, S], BF16, kind="ExternalInput")
    wq = nc.dram_tensor("wq", [DM, QR], BF16, kind="ExternalInput")
    wk = nc.dram_tensor("wk", [DM, HD], BF16, kind="ExternalInput")
    wv = nc.dram_tensor("wv", [DM, HD], BF16, kind="ExternalInput")
    wo = nc.dram_tensor("wo", [QR, DM], BF16, kind="ExternalInput")
    cosT = nc.dram_tensor("cosT", [128, S], BF16, kind="ExternalInput")
    sinT = nc.dram_tensor("sinT", [128, S], BF16, kind="ExternalInput")
    permT = nc.dram_tensor("permT", [128, 128], BF16, kind="ExternalInput")
    tri = nc.dram_tensor("tri", [128, 128], BF16, kind="ExternalInput")
    identd = nc.dram_tensor("ident", [64, 64], BF16, kind="ExternalInput")
    out = nc.dram_tensor("out", [ORS, DM], F32, kind="ExternalOutput")

    groups = [[0, 1, 2, 3], [4, 5, 6, 7]]
    SCB = SCB_CFG   # "sc" psum slots ([128,1024] f32 = 2 banks each)
    ATB = ATB_CFG   # "attn" psum slots
    WB = 2 if XB2 else 1

    with tile.TileContext(nc) as tc:
        with tc.tile_pool(name="const", bufs=1) as constp, \
             tc.tile_pool(name="pers", bufs=1) as pers, \
             tc.tile_pool(name="work", bufs=1) as work, \
             tc.tile_pool(name="ps", bufs=1, space="PSUM") as psp, \
             tc.tile_pool(name="dram", bufs=1, space="DRAM") as dramp:

            # ---- constants (already bf16 in DRAM)
            def load_const(dram_t, rows, cols, cname, dt_out=BF16):
                t = constp.tile([rows, cols], dt_out, name=cname, tag=cname)
                nc.sync.dma_start(t[:], dram_t.ap())
                return t

            cos_sb = load_const(cosT, 128, S, "cos_sb")
            sin_sb = load_const(sinT, 128, S, "sin_sb")
            perm_sb = load_const(permT, 128, 128, "perm_sb")
            tri_sb = load_const(tri, 128, 128, "tri_sb")
            ident128 = constp.tile([128, 64], BF16, name="ident128",
                                   tag="ident128")
            nc.sync.dma_start(ident128[0:64, :], identd.ap())
            nc.sync.dma_start(ident128[64:128, :], identd.ap())

            # weights (once): per 128-row dm chunk for qkv
            WCOLS = QR + HD + HD
            wq_sb, wkv_sb = [], []
            for c in range(8):
                wt = pers.tile([128, WCOLS], BF16, name=f"w_{c}",
                               tag=f"w_{c}")
                nc.sync.dma_start(wt[:, 0:QR],
                                  wq.ap()[128 * c:128 * c + 128, :])
                nc.sync.dma_start(wt[:, QR:QR + HD],
                                  wk.ap()[128 * c:128 * c + 128, :])
                nc.sync.dma_start(wt[:, QR + HD:QR + 2 * HD],
                                  wv.ap()[128 * c:128 * c + 128, :])
                wq_sb.append(wt[:, 0:QR])
                wkv_sb.append(wt[:, QR:QR + 2 * HD])
            # wo rows [256 local ao, 1024 cols] -> 2 chunks of 128
            wo_sb = []
            for c2 in range(2):
                wt = pers.tile([128, DM], BF16, name=f"wo_{c2}",
                               tag=f"wo_{c2}")
                nc.sync.dma_start(wt[:], wo.ap()[128 * c2:128 * c2 + 128, :])
                wo_sb.append(wt)

            RSDT = BF16 if RS_BF16 else F32

            def load_x():
                tiles = []
                for c in range(8):
                    t = work.tile([128, S], BF16, tag=f"xbf_{c}", bufs=1,
                                  name=f"xbf_{c}")
                    nc.sync.dma_start(
                        t[:], xT.ap()[128 * c:128 * c + 128, :])
                    tiles.append(t)
                return tiles

            ones64 = constp.tile([1, 64], BF16, name="ones64", tag="ones64")
            nc.vector.memset(ones64[:], 1.0)

            for _rep in range(nrep):
                rs_in = [dramp.tile([1024, DM], RSDT, name=f"rs_in{qh}",
                                    tag=f"rs_in{qh}", bufs=2)
                         for qh in range(2)]
                rs_out = [dramp.tile([256, DM], RSDT, name=f"rs_out{qh}",
                                     tag=f"rs_out{qh}", bufs=2)
                          for qh in range(2)]
                xbf = load_x()
                v_aug = work.tile([128, 16 * (HD + 1)], BF16, tag="vaug",
                                  bufs=WB)

                # ---- projections (transposed outputs) + RoPE
                q_raw = [work.tile([128, S], BF16, name=f"qraw_{rc}",
                                   tag=f"qraw_{rc}", bufs=WB)
                         for rc in range(2)]
                q_rot = [work.tile([128, S], BF16, name=f"qrot_{rc}",
                                   tag=f"qrot_{rc}", bufs=WB)
                         for rc in range(2)]
                k_raw = work.tile([64, S], BF16, tag="kraw", bufs=WB)
                k_rot = work.tile([128, S], BF16, tag="krot", bufs=WB)
                vT_sb = work.tile([64, S], BF16, tag="vT", bufs=WB)

                # kv proj (merged: wk|wv adjacent -> kT rows 0:64, vT rows
                # 64:128 of one psum tile) + k rope + V transposes first so
                # attention can start as soon as the first q half is rotated
                nc.vector.memset(v_aug[:], 1.0)
                for t4 in range(4):
                    sl = slice(512 * t4, 512 * t4 + 512)
                    ps = psp.tile([128, 512], F32, tag="sc", bufs=SCB)
                    for c in range(8):
                        nc.tensor.matmul(ps[:], wkv_sb[c],
                                         xbf[c][:, sl],
                                         start=(c == 0), stop=(c == 7))
                    nc.vector.tensor_copy(k_raw[:, sl], ps[0:64, :])
                    nc.vector.tensor_copy(vT_sb[:, sl], ps[64:128, :])
                    # k rope for this chunk
                    sw = psp.tile([64, 512], F32, tag="sc", bufs=SCB)
                    nc.tensor.matmul(sw[:], perm_sb[0:64, 0:64],
                                     k_raw[:, sl], start=True, stop=True)
                    t1 = work.tile([64, 512], BF16, tag="t1k", bufs=2)
                    nc.vector.tensor_mul(t1[:], k_raw[:, sl],
                                         cos_sb[0:64, sl])
                    t2 = work.tile([64, 512], BF16, tag="t2k", bufs=2)
                    nc.vector.tensor_mul(t2[:], sw[:], sin_sb[0:64, sl])
                    nc.vector.tensor_add(k_rot[0:64, sl], t1[:], t2[:])
                    nc.sync.dma_start(k_rot[64:128, sl], k_rot[0:64, sl])
                    # V transposes for this chunk (4 k-blocks)
                    for j in range(4 * t4, 4 * t4 + 4):
                        tp = psp.tile([128, 64], BF16, tag="sc", bufs=SCB)
                        nc.tensor.transpose(
                            tp[:],
                            vT_sb[:, 128 * j:128 * j + 128],
                            ident128[0:64, :])
                        nc.vector.tensor_copy(v_aug[:, 65 * j:65 * j + 64],
                                               tp[:])

                # q proj + rope, one 128-row chunk (= 2 heads) at a time
                def emit_qproj(rc, t4s):
                    for t4 in t4s:
                        sl = slice(512 * t4, 512 * t4 + 512)
                        ps = psp.tile([128, 512], F32, tag="sc", bufs=SCB)
                        for c in range(8):
                            nc.tensor.matmul(
                                ps[:],
                                wq_sb[c][:, 128 * rc:128 * rc + 128],
                                xbf[c][:, sl],
                                start=(c == 0), stop=(c == 7))
                        nc.vector.tensor_copy(q_raw[rc][:, sl], ps[:])
                        sw = psp.tile([128, 512], F32, tag="sc", bufs=SCB)
                        nc.tensor.matmul(sw[:], perm_sb[:], q_raw[rc][:, sl],
                                         start=True, stop=True)
                        t1 = work.tile([128, 512], BF16, tag="t1", bufs=2)
                        nc.vector.tensor_mul(t1[:], q_raw[rc][:, sl],
                                             cos_sb[:, sl])
                        t2 = work.tile([128, 512], BF16, tag="t2", bufs=2)
                        nc.vector.tensor_mul(t2[:], sw[:], sin_sb[:, sl])
                        nc.vector.tensor_add(q_rot[rc][:, sl], t1[:], t2[:])

                # normalized attention probs, kept in SBUF:
                # attq[qh][p][64*s : 64*s+64, :] = head h=2p+s, tokens of
                # half qh, layout [ao, tok]
                attq = [[work.tile([128, 1024], BF16, name=f"att{qh}{p}",
                                   tag=f"att{qh}{p}")
                         for p in range(2)] for qh in range(2)]

                # ---- attention: one head over one 1024-token half
                def emit_head(qh, h):
                    jmax = 8 * (qh + 1)
                    hb = 64 * (h % 2)
                    q_h = q_rot[h // 2]
                    attn_ps = psp.tile([65, 1024], F32, tag="attn", bufs=ATB)

                    def emit_pv(pv):
                        pt_, q0_, j_ = pv
                        for r in range(2):
                            rs = 1024 * qh + 512 * r
                            s0 = max(q0_, rs)
                            s1 = rs + 512
                            if s0 >= s1:
                                continue
                            nc.tensor.matmul(
                                attn_ps[:, s0 - 1024 * qh:
                                        s1 - 1024 * qh],
                                v_aug[:, 65 * j_:65 * j_ + 65],
                                pt_[:, s0 - q0_:s1 - q0_],
                                start=(j_ == 0),
                                stop=(j_ == 8 * qh + 4 * r + 3))

                    pend = []
                    for j in range(jmax):
                        q0 = max(1024 * qh, 128 * j)
                        q1 = 1024 * (qh + 1)
                        qlen = q1 - q0
                        sc = psp.tile([128, 1024], F32, tag="sc", bufs=SCB)
                        off = 0
                        while off < qlen:
                            n = min(512, qlen - off)
                            nc.tensor.matmul(
                                sc[:, off:off + n],
                                k_rot[hb:hb + 64,
                                      128 * j:128 * j + 128],
                                q_h[hb:hb + 64,
                                    q0 + off:q0 + off + n],
                                start=True, stop=True)
                            off += n
                        pt = work.tile([128, 1024], BF16, tag="pt",
                                       bufs=PTB)
                        nc.scalar.activation(pt[:, 0:qlen],
                                             sc[:, 0:qlen],
                                             FP.Exp, scale=0.125)
                        if 128 * j >= 1024 * qh:
                            eng = nc.gpsimd if TRI_POOL else nc.vector
                            eng.tensor_mul(pt[:, 0:128],
                                           pt[:, 0:128],
                                           tri_sb[:])
                        pend.append((pt, q0, j))
                        if len(pend) >= PEND:
                            emit_pv(pend.pop(0))
                    for pv in pend:
                        emit_pv(pv)
                    # evacuate PSUM early (frees the single attn slot),
                    # then normalize off the critical path:
                    # att = attn[0:64] * bcast(1/Z)
                    p, s_ = h // 2, h % 2
                    dst = attq[qh][p][64 * s_:64 * s_ + 64, :]
                    nc.vector.tensor_copy(dst, attn_ps[0:64, :])
                    zr1 = work.tile([1, 1024], BF16, tag="zr1", bufs=2)
                    with nc.allow_low_precision(
                            reason="bf16 1/Z; rel-err budget 2e-2"):
                        nc.vector.reciprocal(zr1[:], attn_ps[64:65, :])
                    if ZBC_PE:
                        # broadcast 1/Z across 64 partitions with a
                        # contraction-1 matmul (ones column x zr1 row); the
                        # normalize then reads straight from PSUM -- no
                        # DRAM roundtrip on the epilogue critical path
                        for chh in range(2):
                            csl2 = slice(512 * chh, 512 * chh + 512)
                            zbc = psp.tile([64, 512], F32, tag="sc",
                                           bufs=SCB, name="zbc")
                            nc.tensor.matmul(zbc[:], ones64[:],
                                             zr1[:, csl2],
                                             start=True, stop=True)
                            with nc.allow_low_precision(
                                    reason="bf16 softmax normalize"):
                                nc.vector.tensor_mul(
                                    dst[:, csl2], dst[:, csl2], zbc[:])
                    else:
                        zdram = dramp.tile([1, 1024], BF16, tag="zdram",
                                           name="zdram", bufs=2)
                        zr = work.tile([128, 1024], BF16, tag="zr", bufs=2)
                        zsl = zr[64 * s_:64 * s_ + 64, :]
                        nc.sync.dma_start(zdram[:], zr1[:])
                        nc.sync.dma_start(
                            zsl, zdram.partition_broadcast(64).squeeze(1))
                        with nc.allow_low_precision(
                                reason="bf16 softmax normalize"):
                            nc.vector.tensor_mul(dst, dst, zsl)

                # ---- row-sharded o-projection partial for one token half
                def emit_oproj(qh):
                    for tc8 in range(8):
                        csl = slice(128 * tc8, 128 * tc8 + 128)
                        po = psp.tile([128, 1024], F32, tag="sc", bufs=SCB)
                        for ch in range(2):
                            osl = slice(512 * ch, 512 * ch + 512)
                            for p in range(2):
                                nc.tensor.matmul(
                                    po[:, osl],
                                    attq[qh][p][:, csl],
                                    wo_sb[p][:, osl],
                                    start=(p == 0), stop=(p == 1))
                        ot = work.tile([128, 1024], RSDT, tag="ot", bufs=3)
                        nc.vector.tensor_copy(ot[:], po[:])
                        nc.sync.dma_start(
                            rs_in[qh][csl, :], ot[:])

                def emit_rs(qh):
                    # NB: collectives cannot write IO tensors (walrus
                    # checkCollective) -> RS lands in rs_out, DMA'd to out.
                    nc.gpsimd.collective_compute(
                        "ReduceScatter", mybir.AluOpType.add,
                        replica_groups=groups,
                        ins=[rs_in[qh].opt()],
                        outs=[rs_out[qh].opt()])

                def emit_rs_out(qh):
                    # RS result (internal DRAM) -> f32 ExternalOutput.
                    # Issued from the (otherwise idle) Pool queue: on the
                    # in-order SP queue its wait-for-RS would head-block the
                    # next iteration's x loads.
                    if not RS_BF16:
                        # column-split so the DRAM->DRAM copy keeps 2KB
                        # descriptors (a fully-contiguous AP collapses to one
                        # descriptor on one DMA engine)
                        for ch in range(2):
                            osl = slice(512 * ch, 512 * ch + 512)
                            nc.gpsimd.dma_start(
                                out.ap()[256 * qh:256 * qh + 256, osl],
                                rs_out[qh][:, osl])
                        return
                    for c2 in range(2):
                        t = work.tile([128, DM], BF16, tag="rso", bufs=2)
                        nc.gpsimd.dma_start(
                            t[:], rs_out[qh][128 * c2:128 * c2 + 128, :])
                        tf = work.tile([128, DM], F32, tag="rsof", bufs=2)
                        nc.vector.tensor_copy(tf[:], t[:])
                        nc.gpsimd.dma_start(
                            out.ap()[256 * qh + 128 * c2:
                                     256 * qh + 128 * c2 + 128, :],
                            tf[:])

                emit_qproj(0, [0, 1])
                emit_qproj(1, [0, 1])
                emit_head(0, 0)
                emit_head(0, 1)
                emit_qproj(0, [2, 3])
                emit_qproj(1, [2, 3])
                emit_head(0, 2)
                emit_head(0, 3)
                if _rep + 1 < nrep:
                    xbf_next = load_x()
                emit_head(1, 0)
                emit_oproj(0)
                emit_rs(0)
                emit_head(1, 1)
                emit_head(1, 2)
                emit_rs_out(0)
                emit_head(1, 3)
                emit_oproj(1)
                emit_rs(1)
                emit_rs_out(1)

    nc.compile()
    return nc


def _prep_inputs(x, cos, sin, wq, wk, wv, wo):
    x = np.ascontiguousarray(x, np.float32)
    cos = np.asarray(cos, np.float32)
    sin = np.asarray(sin, np.float32)
    wq = np.asarray(wq, np.float32)
    wk = np.asarray(wk, np.float32)
    wv = np.asarray(wv, np.float32)
    wo = np.asarray(wo, np.float32)

    sinp = np.concatenate([-sin[:, :HD // 2], sin[:, HD // 2:]], axis=1)
    cosT_np = np.ascontiguousarray(np.tile(cos.T, (2, 1)))        # [128, S]
    sinT_np = np.ascontiguousarray(np.tile(sinp.T, (2, 1)))       # [128, S]
    perm = np.zeros((128, 128), np.float32)
    for i in range(128):
        perm[i, (i + 32) % 64 + 64 * (i // 64)] = 1.0
    permT_np = np.ascontiguousarray(perm.T)
    tri_np = (np.arange(128)[:, None] <= np.arange(128)[None, :]) \
        .astype(np.float32)

    BFN = ml_dtypes.bfloat16
    in_maps = []
    for c in range(NCORES):
        b, g = c // TPG, c % TPG
        in_maps.append({
            "xT": np.ascontiguousarray(x[b].T).astype(BFN),
            "wq": np.ascontiguousarray(wq[:, QR * g:QR * (g + 1)]).astype(BFN),
            "wk": np.ascontiguousarray(wk[:, HD * g:HD * (g + 1)]).astype(BFN),
            "wv": np.ascontiguousarray(wv[:, HD * g:HD * (g + 1)]).astype(BFN),
            "wo": np.ascontiguousarray(wo[QR * g:QR * (g + 1), :]).astype(BFN),
            "cosT": cosT_np.astype(BFN),
            "sinT": sinT_np.astype(BFN),
            "permT": permT_np.astype(BFN),
            "tri": tri_np.astype(BFN),
            "ident": np.eye(64, dtype=BFN),
        })
    return in_maps


def kernel(x, cos, sin, wq, wk, wv, wo):
    global LAST_RESULTS
    if "nc" not in _CACHE:
        _CACHE["nc"] = _build()
    nc = _CACHE["nc"]
    in_maps = _prep_inputs(x, cos, sin, wq, wk, wv, wo)
    res = bass_utils.run_bass_kernel_spmd(
        nc, in_maps, core_ids=list(range(NCORES)), trace=PROFILE)
    LAST_RESULTS = res
    outs = [res.results[c]["out"] for c in range(NCORES)]
    full = np.empty((B, S, DM), np.float32)
    for b in range(B):
        for g in range(TPG):
            o = outs[TPG * b + g]
            full[b, 256 * g:256 * g + 256] = o[0:256]
            full[b, 1024 + 256 * g:1024 + 256 * g + 256] = o[256:512]
    return full


# revision 46
# speedup vs baseline: 1.3724x; 1.3724x over previous
"""Distributed GQA attention kernel for 8 TRN2 NeuronCores (Bass/Tile).

Problem (hardcoded): B=2, S=2048, DM=1024, H=16 q-heads, KH=4 kv-heads, HD=64.
reference: out = softmax_causal((RoPE(x@wq) @ RoPE(x@wk)^T)/sqrt(HD)) @ (x@wv) @ wo

Sharding: core c in 0..7 -> batch b = c//4, kv-group g = c%4.
Each core computes q-heads [4g..4g+4) and kv head g for its batch, keeps the
normalized attention probabilities in SBUF, row-shards the o-projection
(wo rows [256g:256g+256)), and the 4 cores of one batch ReduceScatter(add)
their full-width o-proj partials.  Core g ends up with token rows
[256g:256g+256) of each 1024-token half; the host reassembles the full
[2,2048,1024] output from the 8 per-core [512,1024] slices.

The ReduceScatter replaces the baseline AllGather + column-sharded o-proj:
the collective cost scales with its *output* size, so RS (0.5-2 MiB out)
beats AG (4 MiB out), and splitting it per 1024-token half lets the first
RS run concurrently with the second half's attention compute.

All matmuls run in bf16 with f32 PSUM accumulation.  Scores are computed
transposed ([k,q]) so the softmax denominator falls out of a ones-column in
the PV matmul; softmax skips max-subtraction (scores are O(3) for this
problem scale, well within fp32 exp range).  RoPE's rotate_half is a
permutation matmul; causality is handled by issuing score matmuls only for
q >= k plus one triangular mask multiply on diagonal 128x128 blocks.

_build(nrep=N) repeats the whole per-iteration body N times inside one NEFF
(used by the benchmark to amortize the ~80 ms axon dispatch overhead and
measure per-iteration HW time from the slope).
"""

import numpy as np
import ml_dtypes

import concourse.bass as bass
import concourse.bacc as bacc
import concourse.mybir as mybir
import concourse.tile as tile
from concourse import bass_utils

B, S, DM = 2, 2048, 1024
H, KH, HD = 16, 4, 64
NCORES = 8
TPG = 4            # tensor-parallel group size (cores per batch)
QH_PER_CORE = 4    # q heads per core
QR = QH_PER_CORE * HD   # 256 q rows per core
ORS = S // TPG     # 512 out token rows per core (256 per half)

F32 = mybir.dt.float32
BF16 = mybir.dt.bfloat16
FP = mybir.ActivationFunctionType

_CACHE = {}
PROFILE = False
LAST_RESULTS = None

RS_BF16 = False   # ReduceScatter payload dtype (False -> f32)
XB2 = False       # double-buffer x/q/k/v tiles for cross-iteration overlap
SCB_CFG = 3       # "sc" psum slots
ATB_CFG = 1       # "attn" psum slots
PEND = 4          # PV lookahead depth
PTB = 6           # pt buffer count
TRI_POOL = False  # tri-mask multiplies on gpsimd instead of DVE
PBCAST = False    # zr broadcast via gpsimd.partition_broadcast (no DMA)
ZBC_PE = False    # 1/Z broadcast via PE ones-column matmul (no DMA roundtrip)


def _build(nrep=1):
    nc = bacc.Bacc("TRN2", debug=False, enable_asserts=False,
                   num_devices=NCORES)

    xT = nc.dram_tensor("xT", [DM, S], BF16, kind="ExternalInput")
    wq = nc.dram_tensor("wq", [DM, QR], BF16, kind="ExternalInput")
    wk = nc.dram_tensor("wk", [DM, HD], BF16, kind="ExternalInput")
    wv = nc.dram_tensor("wv", [DM, HD], BF16, kind="ExternalInput")
    wo = nc.dram_tensor("wo", [QR, DM], BF16, kind="ExternalInput")
    cosT = nc.dram_tensor("cosT", [128, S], BF16, kind="ExternalInput")
    sinT = nc.dram_tensor("sinT", [128, S], BF16, kind="ExternalInput")
    permT = nc.dram_tensor("permT", [128, 128], BF16, kind="ExternalInput")
    tri = nc.dram_tensor("tri", [128, 128], BF16, kind="ExternalInput")
    identd = nc.dram_tensor("ident", [64, 64], BF16, kind="ExternalInput")
    out = nc.dram_tensor("out", [ORS, DM], F32, kind="ExternalOutput")

    groups = [[0, 1, 2, 3], [4, 5, 6, 7]]
    SCB = SCB_CFG   # "sc" psum slots ([128,1024] f32 = 2 banks each)
    ATB = ATB_CFG   # "attn" psum slots
    WB = 2 if XB2 else 1

    with tile.TileContext(nc) as tc:
        with tc.tile_pool(name="const", bufs=1) as constp, \
             tc.tile_pool(name="pers", bufs=1) as pers, \
             tc.tile_pool(name="work", bufs=1) as work, \
             tc.tile_pool(name="ps", bufs=1, space="PSUM") as psp, \
             tc.tile_pool(name="dram", bufs=1, space="DRAM") as dramp:

            # ---- constants (already bf16 in DRAM)
            def load_const(dram_t, rows, cols, cname, dt_out=BF16):
                t = constp.tile([rows, cols], dt_out, name=cname, tag=cname)
                nc.sync.dma_start(t[:], dram_t.ap())
                return t

            cos_sb = load_const(cosT, 128, S, "cos_sb")
            sin_sb = load_const(sinT, 128, S, "sin_sb")
            perm_sb = load_const(permT, 128, 128, "perm_sb")
            tri_sb = load_const(tri, 128, 128, "tri_sb")
            ident128 = constp.tile([128, 64], BF16, name="ident128",
                                   tag="ident128")
            nc.sync.dma_start(ident128[0:64, :], identd.ap())
            nc.sync.dma_start(ident128[64:128, :], identd.ap())

            # weights (once): per 128-row dm chunk for qkv
            WCOLS = QR + HD + HD
            wq_sb, wkv_sb = [], []
            for c in range(8):
                wt = pers.tile([128, WCOLS], BF16, name=f"w_{c}",
                               tag=f"w_{c}")
                nc.sync.dma_start(wt[:, 0:QR],
                                  wq.ap()[128 * c:128 * c + 128, :])
                nc.sync.dma_start(wt[:, QR:QR + HD],
                                  wk.ap()[128 * c:128 * c + 128, :])
                nc.sync.dma_start(wt[:, QR + HD:QR + 2 * HD],
                                  wv.ap()[128 * c:128 * c + 128, :])
                wq_sb.append(wt[:, 0:QR])
                wkv_sb.append(wt[:, QR:QR + 2 * HD])
            # wo rows [256 local ao, 1024 cols] -> 2 chunks of 128
            wo_sb = []
            for c2 in range(2):
                wt = pers.tile([128, DM], BF16, name=f"wo_{c2}",
                               tag=f"wo_{c2}")
                nc.sync.dma_start(wt[:], wo.ap()[128 * c2:128 * c2 + 128, :])
                wo_sb.append(wt)

            RSDT = BF16 if RS_BF16 else F32

            # ---- x (transposed layout [dm, tokens]), one DMA per chunk.
            # Double-buffered and prefetched: the loads for rep r+1 are
            # emitted mid-rep r (see below) so they sit in the SP queue ahead
            # of the qh1 heads' zdram chains, which would otherwise delay
            # them past the next iteration's kv projection.
            def load_x():
                tiles = []
                for c in range(8):
                    t = work.tile([128, S], BF16, tag=f"xbf_{c}", bufs=2,
                                  name=f"xbf_{c}")
                    nc.sync.dma_start(
                        t[:], xT.ap()[128 * c:128 * c + 128, :])
                    tiles.append(t)
                return tiles

            # v_aug is persistent: the ones-columns (65j+64) are written
            # once; the V data columns are fully overwritten every rep
            v_aug = pers.tile([128, 16 * (HD + 1)], BF16, tag="vaug",
                              name="v_aug")
            nc.vector.memset(v_aug[:], 1.0)
            ones64 = constp.tile([1, 64], BF16, name="ones64", tag="ones64")
            nc.vector.memset(ones64[:], 1.0)

            xbf_next = load_x()

            for _rep in range(nrep):
                rs_in = [dramp.tile([1024, DM], RSDT, name=f"rs_in{qh}",
                                    tag=f"rs_in{qh}", bufs=2)
                         for qh in range(2)]
                rs_out = [dramp.tile([256, DM], RSDT, name=f"rs_out{qh}",
                                     tag=f"rs_out{qh}", bufs=2)
                          for qh in range(2)]
                xbf = xbf_next

                # ---- projections (transposed outputs) + RoPE
                q_raw = [work.tile([128, S], BF16, name=f"qraw_{rc}",
                                   tag=f"qraw_{rc}", bufs=WB)
                         for rc in range(2)]
                q_rot = [work.tile([128, S], BF16, name=f"qrot_{rc}",
                                   tag=f"qrot_{rc}", bufs=WB)
                         for rc in range(2)]
                k_raw = work.tile([64, S], BF16, tag="kraw", bufs=WB)
                k_rot = work.tile([128, S], BF16, tag="krot", bufs=WB)
                vT_sb = work.tile([64, S], BF16, tag="vT", bufs=WB)

                # kv proj (merged: wk|wv adjacent -> kT rows 0:64, vT rows
                # 64:128 of one psum tile) + k rope + V transposes first so
                # attention can start as soon as the first q half is rotated
                nc.vector.memset(v_aug[:], 1.0)
                for t4 in range(4):
                    sl = slice(512 * t4, 512 * t4 + 512)
                    ps = psp.tile([128, 512], F32, tag="sc", bufs=SCB)
                    for c in range(8):
                        nc.tensor.matmul(ps[:], wkv_sb[c],
                                         xbf[c][:, sl],
                                         start=(c == 0), stop=(c == 7))
                    nc.vector.tensor_copy(k_raw[:, sl], ps[0:64, :])
                    nc.vector.tensor_copy(vT_sb[:, sl], ps[64:128, :])
                    # k rope for this chunk
                    sw = psp.tile([64, 512], F32, tag="sc", bufs=SCB)
                    nc.tensor.matmul(sw[:], perm_sb[0:64, 0:64],
                                     k_raw[:, sl], start=True, stop=True)
                    t1 = work.tile([64, 512], BF16, tag="t1k", bufs=2)
                    nc.vector.tensor_mul(t1[:], k_raw[:, sl],
                                         cos_sb[0:64, sl])
                    t2 = work.tile([64, 512], BF16, tag="t2k", bufs=2)
                    nc.vector.tensor_mul(t2[:], sw[:], sin_sb[0:64, sl])
                    nc.vector.tensor_add(k_rot[0:64, sl], t1[:], t2[:])
                    nc.sync.dma_start(k_rot[64:128, sl], k_rot[0:64, sl])
                    # V transposes for this chunk (4 k-blocks)
                    for j in range(4 * t4, 4 * t4 + 4):
                        tp = psp.tile([128, 64], BF16, tag="sc", bufs=SCB)
                        nc.tensor.transpose(
                            tp[:],
                            vT_sb[:, 128 * j:128 * j + 128],
                            ident128[0:64, :])
                        nc.vector.tensor_copy(v_aug[:, 65 * j:65 * j + 64],
                                               tp[:])

                # q proj + rope, one 128-row chunk (= 2 heads) at a time
                def emit_qproj(rc, t4s):
                    for t4 in t4s:
                        sl = slice(512 * t4, 512 * t4 + 512)
                        ps = psp.tile([128, 512], F32, tag="sc", bufs=SCB)
                        for c in range(8):
                            nc.tensor.matmul(
                                ps[:],
                                wq_sb[c][:, 128 * rc:128 * rc + 128],
                                xbf[c][:, sl],
                                start=(c == 0), stop=(c == 7))
                        nc.vector.tensor_copy(q_raw[rc][:, sl], ps[:])
                        sw = psp.tile([128, 512], F32, tag="sc", bufs=SCB)
                        nc.tensor.matmul(sw[:], perm_sb[:], q_raw[rc][:, sl],
                                         start=True, stop=True)
                        t1 = work.tile([128, 512], BF16, tag="t1", bufs=2)
                        nc.vector.tensor_mul(t1[:], q_raw[rc][:, sl],
                                             cos_sb[:, sl])
                        t2 = work.tile([128, 512], BF16, tag="t2", bufs=2)
                        nc.vector.tensor_mul(t2[:], sw[:], sin_sb[:, sl])
                        nc.vector.tensor_add(q_rot[rc][:, sl], t1[:], t2[:])

                # normalized attention probs, kept in SBUF:
                # attq[qh][p][64*s : 64*s+64, :] = head h=2p+s, tokens of
                # half qh, layout [ao, tok]
                attq = [[work.tile([128, 1024], BF16, name=f"att{qh}{p}",
                                   tag=f"att{qh}{p}")
                         for p in range(2)] for qh in range(2)]

                # ---- attention: one head over one 1024-token half
                def emit_head(qh, h):
                    jmax = 8 * (qh + 1)
                    hb = 64 * (h % 2)
                    q_h = q_rot[h // 2]
                    attn_ps = psp.tile([65, 1024], F32, tag="attn", bufs=ATB)

                    def emit_pv(pv):
                        pt_, q0_, j_ = pv
                        for r in range(2):
                            rs = 1024 * qh + 512 * r
                            s0 = max(q0_, rs)
                            s1 = rs + 512
                            if s0 >= s1:
                                continue
                            nc.tensor.matmul(
                                attn_ps[:, s0 - 1024 * qh:
                                        s1 - 1024 * qh],
                                v_aug[:, 65 * j_:65 * j_ + 65],
                                pt_[:, s0 - q0_:s1 - q0_],
                                start=(j_ == 0),
                                stop=(j_ == 8 * qh + 4 * r + 3))

                    pend = []
                    for j in range(jmax):
                        q0 = max(1024 * qh, 128 * j)
                        q1 = 1024 * (qh + 1)
                        qlen = q1 - q0
                        sc = psp.tile([128, 1024], F32, tag="sc", bufs=SCB)
                        off = 0
                        while off < qlen:
                            n = min(512, qlen - off)
                            nc.tensor.matmul(
                                sc[:, off:off + n],
                                k_rot[hb:hb + 64,
                                      128 * j:128 * j + 128],
                                q_h[hb:hb + 64,
                                    q0 + off:q0 + off + n],
                                start=True, stop=True)
                            off += n
                        pt = work.tile([128, 1024], BF16, tag="pt",
                                       bufs=PTB)
                        nc.scalar.activation(pt[:, 0:qlen],
                                             sc[:, 0:qlen],
                                             FP.Exp, scale=0.125)
                        if 128 * j >= 1024 * qh:
                            eng = nc.gpsimd if TRI_POOL else nc.vector
                            eng.tensor_mul(pt[:, 0:128],
                                           pt[:, 0:128],
                                           tri_sb[:])
                        pend.append((pt, q0, j))
                        if len(pend) >= PEND:
                            emit_pv(pend.pop(0))
                    for pv in pend:
                        emit_pv(pv)
                    # evacuate PSUM early (frees the single attn slot),
                    # then normalize off the critical path:
                    # att = attn[0:64] * bcast(1/Z)
                    p, s_ = h // 2, h % 2
                    dst = attq[qh][p][64 * s_:64 * s_ + 64, :]
                    if h == 3 and qh == 1:
                        # last head of the half: the o-projection is about to
                        # wait on this normalize anyway, so skip the DRAM
                        # broadcast roundtrip -- broadcast 1/Z across the 64
                        # partitions with a contraction-1 matmul, normalize
                        # straight out of PSUM, pipelined per column half
                        nc.vector.tensor_copy(dst, attn_ps[0:64, :])
                        zr1 = work.tile([1, 1024], BF16, tag="zr1", bufs=2)
                        with nc.allow_low_precision(
                                reason="bf16 1/Z; rel-err budget 2e-2"):
                            nc.vector.reciprocal(zr1[:], attn_ps[64:65, :])
                        for chh in range(2):
                            csl2 = slice(512 * chh, 512 * chh + 512)
                            zbc = psp.tile([128, 512], F32, tag="sc",
                                           bufs=SCB, name="zbc")
                            zsl2 = zbc[64 * s_:64 * s_ + 64, :]
                            nc.tensor.matmul(zsl2, ones64[:],
                                             zr1[:, csl2],
                                             start=True, stop=True)
                            with nc.allow_low_precision(
                                    reason="bf16 softmax normalize"):
                                nc.vector.tensor_mul(
                                    dst[:, csl2], dst[:, csl2], zsl2)
                    else:
                        nc.vector.tensor_copy(dst, attn_ps[0:64, :])
                        zr1 = work.tile([1, 1024], BF16, tag="zr1", bufs=2)
                        with nc.allow_low_precision(
                                reason="bf16 1/Z; rel-err budget 2e-2"):
                            nc.vector.reciprocal(zr1[:], attn_ps[64:65, :])
                        zdram = dramp.tile([1, 1024], BF16, tag="zdram",
                                           name="zdram", bufs=2)
                        zr = work.tile([128, 1024], BF16, tag="zr", bufs=2)
                        zsl = zr[64 * s_:64 * s_ + 64, :]
                        nc.sync.dma_start(zdram[:], zr1[:])
                        nc.sync.dma_start(
                            zsl, zdram.partition_broadcast(64).squeeze(1))
                        with nc.allow_low_precision(
                                reason="bf16 softmax normalize"):
                            nc.vector.tensor_mul(dst, dst, zsl)

                # ---- row-sharded o-projection partial for one token half
                def emit_oproj(qh):
                    for tc8 in range(8):
                        csl = slice(128 * tc8, 128 * tc8 + 128)
                        po = psp.tile([128, 1024], F32, tag="sc", bufs=SCB)
                        for ch in range(2):
                            osl = slice(512 * ch, 512 * ch + 512)
                            for p in range(2):
                                nc.tensor.matmul(
                                    po[:, osl],
                                    attq[qh][p][:, csl],
                                    wo_sb[p][:, osl],
                                    start=(p == 0), stop=(p == 1))
                        ot = work.tile([128, 1024], RSDT, tag="ot", bufs=3)
                        nc.vector.tensor_copy(ot[:], po[:])
                        nc.sync.dma_start(
                            rs_in[qh][csl, :], ot[:])

                def emit_rs(qh):
                    # NB: collectives cannot write IO tensors (walrus
                    # checkCollective) -> RS lands in rs_out, DMA'd to out.
                    nc.gpsimd.collective_compute(
                        "ReduceScatter", mybir.AluOpType.add,
                        replica_groups=groups,
                        ins=[rs_in[qh].opt()],
                        outs=[rs_out[qh].opt()])

                def emit_rs_out(qh):
                    # RS result (internal DRAM) -> f32 ExternalOutput.
                    # Issued from the (otherwise idle) Pool queue: on the
                    # in-order SP queue its wait-for-RS would head-block the
                    # next iteration's x loads.
                    if not RS_BF16:
                        # column-split so the DRAM->DRAM copy keeps 2KB
                        # descriptors (a fully-contiguous AP collapses to one
                        # descriptor on one DMA engine)
                        for ch in range(2):
                            osl = slice(512 * ch, 512 * ch + 512)
                            nc.gpsimd.dma_start(
                                out.ap()[256 * qh:256 * qh + 256, osl],
                                rs_out[qh][:, osl])
                        return
                    for c2 in range(2):
                        t = work.tile([128, DM], BF16, tag="rso", bufs=2)
                        nc.gpsimd.dma_start(
                            t[:], rs_out[qh][128 * c2:128 * c2 + 128, :])
                        tf = work.tile([128, DM], F32, tag="rsof", bufs=2)
                        nc.vector.tensor_copy(tf[:], t[:])
                        nc.gpsimd.dma_start(
                            out.ap()[256 * qh + 128 * c2:
                                     256 * qh + 128 * c2 + 128, :],
                            tf[:])

                emit_qproj(0, [0, 1])
                emit_qproj(1, [0, 1])
                emit_head(0, 0)
                emit_head(0, 1)
                emit_qproj(0, [2, 3])
                emit_qproj(1, [2, 3])
                emit_head(0, 2)
                emit_head(0, 3)
                if _rep + 1 < nrep:
                    xbf_next = load_x()
                emit_head(1, 0)
                emit_oproj(0)
                emit_rs(0)
                emit_head(1, 1)
                emit_head(1, 2)
                emit_rs_out(0)
                emit_head(1, 3)
                emit_oproj(1)
                emit_rs(1)
                emit_rs_out(1)

    nc.compile()
    return nc


def _prep_inputs(x, cos, sin, wq, wk, wv, wo):
    x = np.ascontiguousarray(x, np.float32)
    cos = np.asarray(cos, np.float32)
    sin = np.asarray(sin, np.float32)
    wq = np.asarray(wq, np.float32)
    wk = np.asarray(wk, np.float32)
    wv = np.asarray(wv, np.float32)
    wo = np.asarray(wo, np.float32)

    sinp = np.concatenate([-sin[:, :HD // 2], sin[:, HD // 2:]], axis=1)
    cosT_np = np.ascontiguousarray(np.tile(cos.T, (2, 1)))        # [128, S]
    sinT_np = np.ascontiguousarray(np.tile(sinp.T, (2, 1)))       # [128, S]
    perm = np.zeros((128, 128), np.float32)
    for i in range(128):
        perm[i, (i + 32) % 64 + 64 * (i // 64)] = 1.0
    permT_np = np.ascontiguousarray(perm.T)
    tri_np = (np.arange(128)[:, None] <= np.arange(128)[None, :]) \
        .astype(np.float32)

    BFN = ml_dtypes.bfloat16
    in_maps = []
    for c in range(NCORES):
        b, g = c // TPG, c % TPG
        in_maps.append({
            "xT": np.ascontiguousarray(x[b].T).astype(BFN),
            "wq": np.ascontiguousarray(wq[:, QR * g:QR * (g + 1)]).astype(BFN),
            "wk": np.ascontiguousarray(wk[:, HD * g:HD * (g + 1)]).astype(BFN),
            "wv": np.ascontiguousarray(wv[:, HD * g:HD * (g + 1)]).astype(BFN),
            "wo": np.ascontiguousarray(wo[QR * g:QR * (g + 1), :]).astype(BFN),
            "cosT": cosT_np.astype(BFN),
            "sinT": sinT_np.astype(BFN),
            "permT": permT_np.astype(BFN),
            "tri": tri_np.astype(BFN),
            "ident": np.eye(64, dtype=BFN),
        })
    return in_maps


def kernel(x, cos, sin, wq, wk, wv, wo):
    global LAST_RESULTS
    if "nc" not in _CACHE:
        _CACHE["nc"] = _build()
    nc = _CACHE["nc"]
    in_maps = _prep_inputs(x, cos, sin, wq, wk, wv, wo)
    res = bass_utils.run_bass_kernel_spmd(
        nc, in_maps, core_ids=list(range(NCORES)), trace=PROFILE)
    LAST_RESULTS = res
    outs = [res.results[c]["out"] for c in range(NCORES)]
    full = np.empty((B, S, DM), np.float32)
    for b in range(B):
        for g in range(TPG):
            o = outs[TPG * b + g]
            full[b, 256 * g:256 * g + 256] = o[0:256]
            full[b, 1024 + 256 * g:1024 + 256 * g + 256] = o[256:512]
    return full
